# Initial kernel scaffold
#
"""Multi-head attention (B=2, S=2048, D=1024, H=16, causal mask) on 8 trn2
NeuronCores.

Sharding: 2-way data parallel over batch x 4-way tensor parallel over head
groups (4 heads / core).  Core c handles batch c//4, head group c%4.

Everything on-chip lives feature-major ("transposed") so no transposes are
ever needed:
  inputs are uploaded as x^T [D, S] in fp16; Q/K projections produce
  Qh^T/Kh^T [e, t]; scores come out keys-major [k, q]; exp(p) feeds A@V
  directly as the moving operand with V (+ a ones column that makes the
  softmax denominator fall out of the same matmul) stationary; the
  attention output appears as x_att^T [e, q], which is exactly the layout
  the output projection wants.  Each core emits its y^T partial [1024, S]
  and the host sums the 4 partials of each batch group during unshard
  (row-parallel tensor-parallel reduction; on-device collectives are not
  launchable as one 8-replica program through this PJRT path).

Softmax runs unnormalized (inputs are unit-scale gaussians, exp cannot
overflow fp32), heads are processed in pairs whose score matmuls land on
different PE row-tiles, and the output projection of chunk qc is emitted
one head-pair into chunk qc+1 so the tensor engine never waits on the
normalize chain.

Mask is handled generically: the [S,S] mask is classified on the host into
128x128 blocks (zero / one / mixed).  Zero blocks are skipped entirely
(this is what makes causal cost ~half of dense), mixed blocks get a
pattern-multiply after exp with deduplicated patterns uploaded as data.
"""

import os
import sys

import numpy as np

for _p in ("/opt/trn_rl_repo", "/root/.axon_site/_ro/trn_rl_repo"):
    if os.path.isdir(_p) and _p not in sys.path:
        sys.path.append(_p)

import ml_dtypes  # noqa: E402
from contextlib import ExitStack  # noqa: E402

import concourse.bass as bass  # noqa: E402
import concourse.tile as tile  # noqa: E402
from concourse import mybir  # noqa: E402

# ----- problem constants (hardcoded per contract) ---------------------------
B, S, D, H, DK = 2, 2048, 1024, 16, 64
NCORES = 8
TP = 4                      # head-parallel ways (per batch group)
EL = D // TP                # 256 local head dims = 4 heads
HL = H // TP                # 4 local heads
QC = 512                    # query-chunk (columns per attention pass)
NQC = S // QC               # 4
KT = 128                    # key tile (contraction tile for A@V)
NKT = S // KT               # 16
P = 128
NMT = D // P                # 8 output-feature tiles
OWN = D // TP               # 256 output features owned per core after RS
RG = [[0, 1, 2, 3], [4, 5, 6, 7]]
SCALE = 1.0 / np.sqrt(DK)

F32 = mybir.dt.float32
F32R = mybir.dt.float32r
F16 = mybir.dt.float16
F16NP = np.float16
BF16 = mybir.dt.bfloat16
BF16NP = ml_dtypes.bfloat16


# ----- host-side mask analysis ---------------------------------------------
class _KTile:
    __slots__ = ("kt", "s0", "s1", "muls", "first", "last")

    def __init__(self, kt, s0, s1, muls):
        self.kt, self.s0, self.s1, self.muls = kt, s0, s1, muls
        self.first = False
        self.last = False


def _mask_plan(mask2d):
    """mask2d: [S, S] ints, mask2d[q, k] (1 = attend).  Returns
    (plan, patterns) where plan[qc] is a list of _KTile and patterns is a
    bf16 array [n_pat, 128, 128] of transposed (k-major) mask blocks."""
    mT = (mask2d != 0).astype(np.float32).T          # [k, q]
    nqt = S // KT
    # classify [KT x KT] blocks: 0 zero, 1 one, else mixed
    blk = mT.reshape(NKT, KT, nqt, KT).transpose(0, 2, 1, 3)  # [kt, qt, 128, 128]
    sums = blk.sum(axis=(2, 3))
    patterns = []
    pat_idx = {}

    def pattern_id(kt, qt):
        key = blk[kt, qt].tobytes()
        if key not in pat_idx:
            pat_idx[key] = len(patterns)
            patterns.append(blk[kt, qt].astype(np.float16))
        return pat_idx[key]

    qt_per_qc = QC // KT
    plan = []
    for qc in range(NQC):
        tiles = []
        for kt in range(NKT):
            sub = sums[kt, qc * qt_per_qc:(qc + 1) * qt_per_qc]
            nz = [i for i in range(qt_per_qc) if sub[i] > 0]
            if not nz:
                continue
            s0, s1 = nz[0] * KT, (nz[-1] + 1) * KT
            tiles.append(_KTile(kt, s0, s1, None))
        if not tiles:
            raise ValueError(f"query chunk {qc} has no unmasked keys")
        u0 = min(t.s0 for t in tiles)
        u1 = max(t.s1 for t in tiles)
        tiles[0].s0, tiles[0].s1 = u0, u1
        tiles[0].first = True
        tiles[-1].last = True
        for t in tiles:
            muls = []
            for qt in range(t.s0 // KT, t.s1 // KT):
                full = sums[t.kt, qc * qt_per_qc + qt]
                if full != KT * KT:          # zero or mixed -> needs pattern
                    muls.append((qt, pattern_id(t.kt, qc * qt_per_qc + qt)))
            t.muls = muls
        plan.append(tiles)
    pats = np.stack(patterns) if patterns else np.zeros((1, KT, KT), np.float16)
    return plan, pats


def _merge_ranges(ranges):
    """merge sorted [lo, hi) ranges that touch"""
    out = []
    for lo, hi in ranges:
        if out and out[-1][1] == lo:
            out[-1][1] = hi
        else:
            out.append([lo, hi])
    return out


# ----- TileContext with a codegen-safe exit drain ---------------------------
# The stock kernel-tail drain carries one semaphore wait per engine/queue the
# kernel touched; CoreV3 codegen rejects instructions with more than two
# waits ("Too many sync wait commands").  Split the waits across preceding
# sync-engine nops, two per instruction, so the drain itself needs none.
class _TileContext(tile.TileContext):
    def _drain_and_barrier(self, tick_clock, wait_clock):
        from concourse.vector_clock import ScopedClock
        nc = self.nc
        probe = nc.sync.nop()
        wait_clock.add_sem_waits(
            probe.ins, ScopedClock({None: tick_clock.global_clock}))
        si = probe.ins.sync_info
        waits = list(si.on_wait) if si and si.on_wait else []
        if len(waits) > 1:
            probe.ins.sync_info = mybir.SyncInfo(
                on_wait=waits[:1], on_update=list(si.on_update or []))
            for w in waits[1:]:
                n = nc.sync.nop()
                n.ins.sync_info = mybir.SyncInfo(on_wait=[w], on_update=[])
        nc.sync.drain()
        nc.all_engine_barrier()
        assert self.sems is not None
        popped = nc._tile_sem_poison_stack.pop()
        assert popped is self._sem_poison
        nc.clear_and_free_semaphores(list(self.sems.allocated().values()))
        nc.all_engine_barrier()


# The same wait-count limit applies to ordinary engine instructions under
# this walrus build, so after the program is fully built, hoist all but one
# wait of every instruction onto preceding same-engine no-ops.
def _legalize_waits(nc, limit=1):
    for bb in nc.main_func.blocks:
        insts = list(bb.instructions)
        out = []
        for inst in insts:
            si = inst.sync_info
            waits = list(si.on_wait) if si and si.on_wait else []
            if len(waits) > limit:
                for w in waits[:-limit]:
                    nop = mybir.InstNoOp(
                        name=nc.get_next_instruction_name(), ins=[], outs=[])
                    nop.engine = inst.engine
                    nop.sync_info = mybir.SyncInfo(on_wait=[w], on_update=[])
                    nc.register_instruction(nop, overwrite=True)
                    out.append(nop)
                inst.sync_info = mybir.SyncInfo(
                    on_wait=waits[-limit:],
                    on_update=list(si.on_update or []))
            out.append(inst)
        bb.instructions = out


DEBUG_TAPS = False


# ----- the bass program -----------------------------------------------------
def build_program(plan, n_pat):
    nc = bass.Bass(num_devices=NCORES)

    xqT = nc.dram_tensor("xqT", [D, S], F16, kind="ExternalInput")
    xkT = nc.dram_tensor("xkT", [D, S], F16, kind="ExternalInput")
    xvT = nc.dram_tensor("xvT", [D, S], F16, kind="ExternalInput")
    wqT = nc.dram_tensor("wqT", [D, EL], F16, kind="ExternalInput")
    wkT = nc.dram_tensor("wkT", [D, EL], F16, kind="ExternalInput")
    wvT = nc.dram_tensor("wvT", [D, EL], F16, kind="ExternalInput")
    woT = nc.dram_tensor("woT", [EL, D], F16, kind="ExternalInput")
    bq2 = nc.dram_tensor("bq2", [2, P], F32, kind="ExternalInput")
    bk2 = nc.dram_tensor("bk2", [2, P], F32, kind="ExternalInput")
    ybias = nc.dram_tensor("ybias", [NMT, P], F32, kind="ExternalInput")
    pats = nc.dram_tensor("pats", [n_pat, KT, KT], F16, kind="ExternalInput")
    yT = nc.dram_tensor("yT", [D, S], F32, kind="ExternalOutput")
    taps = {}
    if DEBUG_TAPS:
        taps["dQt"] = nc.dram_tensor("dQt", [P, 2, S], F16, kind="ExternalOutput")
        taps["dKt"] = nc.dram_tensor("dKt", [P, 2, S], F16, kind="ExternalOutput")
        taps["dVa"] = nc.dram_tensor("dVa", [P, NKT, HL, DK + 1], F16,
                                     kind="ExternalOutput")
        taps["dPt"] = nc.dram_tensor("dPt", [P, NKT, QC], F16,
                                     kind="ExternalOutput")
        taps["dAv"] = nc.dram_tensor("dAv", [P, QC], F32, kind="ExternalOutput")
        taps["dBc"] = nc.dram_tensor("dBc", [P, QC], F32, kind="ExternalOutput")
        taps["dXt"] = nc.dram_tensor("dXt", [P, 2, QC], F16, kind="ExternalOutput")

    with ExitStack() as ctx:
        tc = ctx.enter_context(_TileContext(nc))
        singles = ctx.enter_context(tc.tile_pool(name="singles", bufs=1))

        # --- persistent SBUF state ---
        wq_sb = singles.tile([P, 8, EL], F16)
        wk_sb = singles.tile([P, 8, EL], F16)
        wv_sb = singles.tile([P, 8, EL], F16)
        wo_sb = singles.tile([P, 2, D], F16)
        nc.sync.dma_start(out=wq_sb[:], in_=wqT.rearrange("(a p) e -> p a e", p=P))
        nc.sync.dma_start(out=wk_sb[:], in_=wkT.rearrange("(a p) e -> p a e", p=P))
        nc.sync.dma_start(out=wv_sb[:], in_=wvT.rearrange("(a p) e -> p a e", p=P))
        nc.sync.dma_start(out=wo_sb[:], in_=woT.rearrange("(a p) m -> p a m", p=P))
        bq_sb = singles.tile([P, 2], F32)
        bk_sb = singles.tile([P, 2], F32)
        yb_sb = singles.tile([P, NMT], F32)
        nc.sync.dma_start(out=bq_sb[:], in_=bq2.rearrange("a p -> p a"))
        nc.sync.dma_start(out=bk_sb[:], in_=bk2.rearrange("a p -> p a"))
        nc.sync.dma_start(out=yb_sb[:], in_=ybias.rearrange("a p -> p a"))
        pat_sb = singles.tile([P, n_pat, KT], F16)
        # touch Exp+Ln early so the activation table set loads during the
        # projection phase instead of stalling the first softmax
        warm = singles.tile([P, 1], F32)
        nc.scalar.activation(out=warm[0:1, :], in_=bq_sb[0:1, 0:1],
                             func=mybir.ActivationFunctionType.Exp)
        nc.scalar.activation(out=warm[0:1, :], in_=warm[0:1, :],
                             func=mybir.ActivationFunctionType.Ln)
        nc.sync.dma_start(out=pat_sb[:], in_=pats.rearrange("n p k -> p n k"))

        Qt = singles.tile([P, 2, S], F16)     # [e-within-tile, e-tile, t]
        Kt = singles.tile([P, 2, S], F16)
        Vaug = singles.tile([P, NKT, HL, DK + 1], F16)  # [t-in-ktile, kt, h, e|1]
        nc.vector.memset(Vaug[:, :, :, DK:DK + 1], 1.0)
        ones_row = singles.tile([P, DK], F16)
        nc.vector.memset(ones_row[0:1, :], 1.0)

        # --- phase 1: projections ---
        with tc.tile_pool(name="xin", bufs=4) as xin, \
             tc.tile_pool(name="pjps", bufs=2, space="PSUM") as pjps:
            for name, xdr, w_sb in (("q", xqT, wq_sb), ("k", xkT, wk_sb),
                                    ("v", xvT, wv_sb)):
                xr = xdr.rearrange("(a p) t -> p a t", p=P)
                for tci in range(NQC):
                    tsl = slice(tci * QC, (tci + 1) * QC)
                    x_ch = xin.tile([P, 8, QC], F16, tag="xch", name=f"x_{name}{tci}")
                    nc.sync.dma_start(out=x_ch[:], in_=xr[:, :, tsl])
                    if name != "v":
                        dst, b_sb = (Qt, bq_sb) if name == "q" else (Kt, bk_sb)
                        for et in range(2):
                            ps = pjps.tile([P, QC], F32, tag="pj", name=f"ps_{name}{tci}{et}")
                            for ft in range(8):
                                nc.tensor.matmul(
                                    ps[:],
                                    lhsT=w_sb[:, ft, et * P:(et + 1) * P],
                                    rhs=x_ch[:, ft, :],
                                    start=(ft == 0), stop=(ft == 7))
                            nc.vector.tensor_scalar_add(
                                out=dst[:, et, tsl], in0=ps[:],
                                scalar1=b_sb[:, et:et + 1])
                    else:
                        for tt in range(QC // KT):
                            ktg = tci * (QC // KT) + tt
                            ps = pjps.tile([P, EL], F32, tag="pj", name=f"ps_v{ktg}")
                            for ft in range(8):
                                nc.tensor.matmul(
                                    ps[:],
                                    lhsT=x_ch[:, ft, tt * P:(tt + 1) * P],
                                    rhs=wv_sb[:, ft, :],
                                    start=(ft == 0), stop=(ft == 7))
                            for h in range(HL):
                                nc.vector.tensor_copy(
                                    out=Vaug[:, ktg, h, 0:DK],
                                    in_=ps[:, h * DK:(h + 1) * DK])

        # --- phase 2: attention + output projection, per query chunk ---
        spool = ctx.enter_context(tc.tile_pool(name="spool", bufs=2, space="PSUM"))
        avy = ctx.enter_context(tc.tile_pool(name="avy", bufs=2, space="PSUM"))
        ypp = ctx.enter_context(tc.tile_pool(name="ypp", bufs=2, space="PSUM"))
        ptp = ctx.enter_context(tc.tile_pool(name="ptp", bufs=3))
        xtp = ctx.enter_context(tc.tile_pool(name="xtp", bufs=2))
        nrm = ctx.enter_context(tc.tile_pool(name="nrm", bufs=3))
        ysb = ctx.enter_context(tc.tile_pool(name="ysb", bufs=3))
        dbp = ctx.enter_context(tc.tile_pool(name="dbp", bufs=2, space="DRAM"))

        if DEBUG_TAPS:
            nc.sync.dma_start(out=taps["dQt"][:], in_=Qt[:])
            nc.sync.dma_start(out=taps["dKt"][:], in_=Kt[:])
            nc.sync.dma_start(out=taps["dVa"][:], in_=Vaug[:])

        # output projection for chunk qc, emitted one head into chunk qc+1 so
        # the tensor engine never stalls on the normalize chain of chunk qc
        yTr = yT.rearrange("(a p) t -> p a t", p=P)

        def emit_yproj(qc, xTt):
            for mt in range(NMT):
                yp = ypp.tile([P, QC], F32, tag="yp", name=f"yp{qc}{mt}")
                for ct in range(2):
                    nc.tensor.matmul(
                        yp[:],
                        lhsT=wo_sb[:, ct, mt * P:(mt + 1) * P],
                        rhs=xTt[:, ct, :],
                        start=(ct == 0), stop=(ct == 1))
                ys = ysb.tile([P, QC], F32, tag="ys", name=f"ys{qc}{mt}")
                nc.vector.tensor_scalar_add(out=ys[:], in0=yp[:],
                                            scalar1=yb_sb[:, mt:mt + 1])
                nc.sync.dma_start(out=yTr[:, mt, qc * QC:(qc + 1) * QC],
                                  in_=ys[:])

        pending = None        # (qc, xTt) whose y-projection is not yet emitted
        for qc in range(NQC):
            tiles = plan[qc]
            xTt = xtp.tile([P, 2, QC], F16, tag="xT", name=f"xT{qc}")
            def normalize(h, av):
                # xT_h = av[e] / av[ones-row]; 1/denom = exp(-ln(denom)) on
                # ScalarE — same activation table set as the softmax exp.
                # av is copied out of PSUM right away so the accumulator
                # slot frees for the next head pair without waiting on the
                # broadcast round-trip.
                et = h // 2
                cp = nrm.tile([P, QC], F32, tag="cp", name=f"cp{qc}{h}")
                nc.vector.tensor_copy(out=cp[0:DK, :], in_=av[0:DK, :])
                bc = nrm.tile([P, QC], F32, tag="bc", name=f"bc{qc}{h}")
                rc = nrm.tile([P, QC], F32, tag="rc", name=f"rc{qc}{h}")
                nc.scalar.activation(out=rc[DK:DK + 1, :],
                                     in_=av[DK:DK + 1, :],
                                     func=mybir.ActivationFunctionType.Ln)
                nc.scalar.activation(out=bc[DK:DK + 1, :],
                                     in_=rc[DK:DK + 1, :],
                                     func=mybir.ActivationFunctionType.Exp,
                                     scale=-1.0)
                dnb = dbp.tile([1, QC], F32, tag="dnb", name=f"dnb{qc}{h}")
                nc.sync.dma_start(out=dnb[:], in_=bc[DK:DK + 1, :])
                nc.sync.dma_start(out=bc[0:DK, :],
                                  in_=dnb[0:1, :].partition_broadcast(DK))
                if h % 2 == 0:
                    nc.vector.tensor_tensor(
                        out=xTt[0:DK, et, :], in0=cp[0:DK, :], in1=bc[0:DK, :],
                        op=mybir.AluOpType.mult)
                else:
                    tmp = nrm.tile([P, QC], F16, tag="tmp", name=f"tm{qc}{h}")
                    nc.vector.tensor_tensor(
                        out=tmp[0:DK, :], in0=cp[0:DK, :], in1=bc[0:DK, :],
                        op=mybir.AluOpType.mult)
                    nc.sync.dma_start(out=xTt[DK:P, et, :], in_=tmp[0:DK, :])

            for hp in range(HL // 2):
                # head pair (2hp, 2hp+1): head parity picks SBUF partition
                # half, so the two heads' score matmuls land on different
                # PE row-tiles (T0/T8) and can run concurrently.  One psum
                # tile per k-tile holds both heads side by side; the
                # previous k-tile's A@V matmuls run under the exp latency.
                if hp == 1 and pending is not None:
                    emit_yproj(*pending)
                    pending = None
                et = hp
                ptb = ptp.tile([P, NKT, 2, QC], F16, tag="pt",
                               name=f"pt{qc}{hp}")
                ptbf = ptb.rearrange("p a b c -> p (a b c)")
                avs = [avy.tile([P, QC], F32, tag="avy",
                                name=f"av{qc}{2 * hp + hh}")
                       for hh in range(2)]

                def emit_av(ti, t):
                    for hh in range(2):
                        nc.tensor.matmul(
                            avs[hh][0:DK + 1, t.s0:t.s1],
                            lhsT=Vaug[:, t.kt, 2 * hp + hh, :],
                            rhs=ptb[:, ti, hh, t.s0:t.s1],
                            start=t.first, stop=t.last,
                            skip_group_check=True)

                for ti, t in enumerate(tiles):
                    ps = spool.tile([P, 2 * QC], F32, tag="s",
                                    name=f"s{qc}{hp}{ti}")
                    for hh in range(2):
                        po = hh * DK
                        nc.tensor.matmul(
                            ps[:, hh * QC + t.s0:hh * QC + t.s1],
                            lhsT=Kt[po:po + DK, et,
                                    t.kt * KT:(t.kt + 1) * KT],
                            rhs=Qt[po:po + DK, et,
                                   qc * QC + t.s0:qc * QC + t.s1],
                            start=True, stop=True)
                    if ti > 0:
                        emit_av(ti - 1, tiles[ti - 1])
                    # psum col (hh*QC + c) maps to ptb flat col
                    # (ti*2*QC + hh*QC + c): merged ranges stay merged
                    rgs = _merge_ranges([(hh * QC + t.s0, hh * QC + t.s1)
                                         for hh in range(2)])
                    for lo, hi in rgs:
                        nc.scalar.activation(
                            out=ptbf[:, ti * 2 * QC + lo:ti * 2 * QC + hi],
                            in_=ps[:, lo:hi],
                            func=mybir.ActivationFunctionType.Exp,
                            scale=float(SCALE))
                    for hh in range(2):
                        for qt, pid in t.muls:
                            sl = slice(qt * KT, (qt + 1) * KT)
                            nc.vector.tensor_tensor(
                                out=ptb[:, ti, hh, sl],
                                in0=ptb[:, ti, hh, sl],
                                in1=pat_sb[:, pid, :],
                                op=mybir.AluOpType.mult)
                emit_av(len(tiles) - 1, tiles[-1])
                for hh in range(2):
                    normalize(2 * hp + hh, avs[hh])

            if DEBUG_TAPS and qc == 0:
                nc.sync.dma_start(out=taps["dXt"][:], in_=xTt[:])

            pending = (qc, xTt)
        emit_yproj(*pending)

    _legalize_waits(nc)
    return nc


# ----- SPMD runner ----------------------------------------------------------
# run_bass_kernel_spmd's axon path lowers through jax.jit(shard_map(...)),
# which this jax version emits as `call`-indirect HLO that the bass_exec
# compile hook rejects, and a single 8-replica launch isn't reachable from
# here.  Instead: one single-device jit per core (clean single-computation
# HLO), dispatched asynchronously on all 8 cores.  The NEFF is memoized by
# HLO bytes so walrus runs once, not 8 times.
_NEFF_MEMO = {}


def _install_memo_hook():
    import libneuronxla
    from concourse.bass2jax import install_neuronx_cc_hook

    install_neuronx_cc_hook()
    inner = libneuronxla.neuronx_cc
    if getattr(inner, "_is_memo_hook", False):
        return

    def memo_hook(code, code_format, platform_version, file_prefix):
        import hashlib
        key = hashlib.sha256(bytes(code)).hexdigest()
        if key not in _NEFF_MEMO:
            _NEFF_MEMO[key] = inner(code, code_format, platform_version,
                                    file_prefix)
        return _NEFF_MEMO[key]

    memo_hook._is_memo_hook = True
    libneuronxla.neuronx_cc = memo_hook


def run_spmd(nc, in_maps):
    import jax
    from concourse.bass2jax import _bass_exec_p

    _install_memo_hook()
    n_cores = len(in_maps)
    partition_name = (nc.partition_id_tensor.name
                      if nc.partition_id_tensor is not None else None)
    in_names, out_names, out_avals = [], [], []
    for alloc in nc.m.functions[0].allocations:
        if not isinstance(alloc, mybir.MemoryLocationSet):
            continue
        name = alloc.memorylocations[0].name
        if alloc.kind == "ExternalInput":
            if name != partition_name:
                in_names.append(name)
        elif alloc.kind == "ExternalOutput":
            out_names.append(name)
            out_avals.append(jax.core.ShapedArray(
                tuple(alloc.tensor_shape), mybir.dt.np(alloc.dtype)))
    bind_in_names = tuple(in_names +
                          ([partition_name] if partition_name else []))

    def _body(*args):
        return tuple(_bass_exec_p.bind(
            *args, out_avals=tuple(out_avals), in_names=bind_in_names,
            out_names=tuple(out_names), lowering_input_output_aliases=(),
            sim_require_finite=True, sim_require_nnan=True, nc=nc))

    devices = jax.devices()[:n_cores]
    f = jax.jit(_body)
    futs = []
    for c in range(n_cores):
        args = [jax.device_put(np.asarray(in_maps[c][nm]), devices[c])
                for nm in in_names]
        if partition_name:
            args.append(jax.device_put(np.array([[c]], np.uint32), devices[c]))
        futs.append(f(*args))
    return [{nm: np.asarray(futs[c][i]) for i, nm in enumerate(out_names)}
            for c in range(n_cores)]


# ----- host wrapper ---------------------------------------------------------
_CACHE = {}


def _get_program(mask):
    key = mask.tobytes()
    if key not in _CACHE:
        plan, pats = _mask_plan(mask)
        nc = build_program(plan, pats.shape[0])
        _CACHE[key] = (nc, pats)
    return _CACHE[key]


def make_in_maps(q, k, v, mask, wq, bq, wk, bk, wv, bv, wo, bo, pats):
    q, k, v = (np.asarray(a, np.float32) for a in (q, k, v))
    in_maps = []
    for c in range(NCORES):
        b, g = divmod(c, TP)
        sl = slice(g * EL, (g + 1) * EL)
        woT_g = np.ascontiguousarray(wo[:, sl].T)        # [EL, D]
        in_maps.append({
            "xqT": np.ascontiguousarray(q[b].T.astype(F16NP)),
            "xkT": np.ascontiguousarray(k[b].T.astype(F16NP)),
            "xvT": np.ascontiguousarray(v[b].T.astype(F16NP)),
            "wqT": np.ascontiguousarray(wq[sl, :].T.astype(F16NP)),
            "wkT": np.ascontiguousarray(wk[sl, :].T.astype(F16NP)),
            "wvT": np.ascontiguousarray(wv[sl, :].T.astype(F16NP)),
            "woT": woT_g.astype(F16NP),
            "bq2": np.ascontiguousarray(bq[sl].reshape(2, P)),
            "bk2": np.ascontiguousarray(bk[sl].reshape(2, P)),
            "ybias": np.ascontiguousarray(
                (bv[sl].astype(np.float64) @ woT_g.astype(np.float64))
                .astype(np.float32).reshape(NMT, P)),
            "pats": pats,
        })
    return in_maps


def assemble_output(results, bo):
    y = np.empty((B, S, D), np.float32)
    for b in range(B):
        acc = results[b * TP]["yT"].astype(np.float32)
        for g in range(1, TP):
            acc = acc + results[b * TP + g]["yT"]
        y[b] = acc.T + np.asarray(bo, np.float32)[None, :]
    return y


def kernel(q, k, v, mask, wq, bq, wk, bk, wv, bv, wo, bo):
    mask2d = np.asarray(mask).reshape(S, S)
    nc, pats = _get_program(mask2d)
    in_maps = make_in_maps(q, k, v, mask2d, wq, bq, wk, bk, wv, bv, wo, bo, pats)
    return assemble_output(run_spmd(nc, in_maps), bo)



# revision 1
# speedup vs baseline: 1.3993x; 1.3993x over previous
"""Multi-head attention (B=2, S=2048, D=1024, H=16, causal mask) on 8 trn2
NeuronCores.

Sharding: 2-way data parallel over batch x 4-way tensor parallel over head
groups (4 heads / core).  Core c handles batch c//4, head group c%4.

Everything on-chip lives feature-major ("transposed") so no transposes are
ever needed:
  inputs are uploaded as x^T [D, S] in fp16; Q/K projections produce
  Qh^T/Kh^T [e, t]; scores come out keys-major [k, q]; exp(p) feeds A@V
  directly as the moving operand with V (+ a ones column that makes the
  softmax denominator fall out of the same matmul) stationary; the
  attention output appears as x_att^T [e, q], which is exactly the layout
  the output projection wants.  Each core emits its y^T partial [1024, S]
  and the host sums the 4 partials of each batch group during unshard
  (row-parallel tensor-parallel reduction; on-device collectives are not
  launchable as one 8-replica program through this PJRT path).

Softmax runs unnormalized (inputs are unit-scale gaussians, exp cannot
overflow fp32), heads are processed in pairs whose score matmuls land on
different PE row-tiles, and the output projection of chunk qc is emitted
one head-pair into chunk qc+1 so the tensor engine never waits on the
normalize chain.

Mask is handled generically: the [S,S] mask is classified on the host into
128x128 blocks (zero / one / mixed).  Zero blocks are skipped entirely
(this is what makes causal cost ~half of dense), mixed blocks get a
pattern-multiply after exp with deduplicated patterns uploaded as data.
"""

import os
import sys

import numpy as np

for _p in ("/opt/trn_rl_repo", "/root/.axon_site/_ro/trn_rl_repo"):
    if os.path.isdir(_p) and _p not in sys.path:
        sys.path.append(_p)

import ml_dtypes  # noqa: E402
from contextlib import ExitStack  # noqa: E402

import concourse.bass as bass  # noqa: E402
import concourse.tile as tile  # noqa: E402
from concourse import mybir  # noqa: E402

# ----- problem constants (hardcoded per contract) ---------------------------
B, S, D, H, DK = 2, 2048, 1024, 16, 64
NCORES = 8
TP = 4                      # head-parallel ways (per batch group)
EL = D // TP                # 256 local head dims = 4 heads
HL = H // TP                # 4 local heads
QC = 512                    # query-chunk (columns per attention pass)
NQC = S // QC               # 4
KT = 128                    # key tile (contraction tile for A@V)
NKT = S // KT               # 16
P = 128
NMT = D // P                # 8 output-feature tiles
OWN = D // TP               # 256 output features owned per core after RS
RG = [[0, 1, 2, 3], [4, 5, 6, 7]]
SCALE = 1.0 / np.sqrt(DK)

F32 = mybir.dt.float32
F32R = mybir.dt.float32r
F16 = mybir.dt.float16
F16NP = np.float16
BF16 = mybir.dt.bfloat16
BF16NP = ml_dtypes.bfloat16


# ----- host-side mask analysis ---------------------------------------------
class _KTile:
    __slots__ = ("kt", "s0", "s1", "muls", "first", "last")

    def __init__(self, kt, s0, s1, muls):
        self.kt, self.s0, self.s1, self.muls = kt, s0, s1, muls
        self.first = False
        self.last = False


def _mask_plan(mask2d):
    """mask2d: [S, S] ints, mask2d[q, k] (1 = attend).  Returns
    (plan, patterns) where plan[qc] is a list of _KTile and patterns is a
    bf16 array [n_pat, 128, 128] of transposed (k-major) mask blocks."""
    mT = (mask2d != 0).astype(np.float32).T          # [k, q]
    nqt = S // KT
    # classify [KT x KT] blocks: 0 zero, 1 one, else mixed
    blk = mT.reshape(NKT, KT, nqt, KT).transpose(0, 2, 1, 3)  # [kt, qt, 128, 128]
    sums = blk.sum(axis=(2, 3))
    patterns = []
    pat_idx = {}

    def pattern_id(kt, qt):
        key = blk[kt, qt].tobytes()
        if key not in pat_idx:
            pat_idx[key] = len(patterns)
            patterns.append(blk[kt, qt].astype(np.float16))
        return pat_idx[key]

    qt_per_qc = QC // KT
    plan = []
    for qc in range(NQC):
        tiles = []
        for kt in range(NKT):
            sub = sums[kt, qc * qt_per_qc:(qc + 1) * qt_per_qc]
            nz = [i for i in range(qt_per_qc) if sub[i] > 0]
            if not nz:
                continue
            s0, s1 = nz[0] * KT, (nz[-1] + 1) * KT
            tiles.append(_KTile(kt, s0, s1, None))
        if not tiles:
            raise ValueError(f"query chunk {qc} has no unmasked keys")
        u0 = min(t.s0 for t in tiles)
        u1 = max(t.s1 for t in tiles)
        tiles[0].s0, tiles[0].s1 = u0, u1
        tiles[0].first = True
        tiles[-1].last = True
        for t in tiles:
            muls = []
            for qt in range(t.s0 // KT, t.s1 // KT):
                full = sums[t.kt, qc * qt_per_qc + qt]
                if full != KT * KT:          # zero or mixed -> needs pattern
                    muls.append((qt, pattern_id(t.kt, qc * qt_per_qc + qt)))
            t.muls = muls
        plan.append(tiles)
    pats = np.stack(patterns) if patterns else np.zeros((1, KT, KT), np.float16)
    return plan, pats


def _merge_ranges(ranges):
    """merge sorted [lo, hi) ranges that touch"""
    out = []
    for lo, hi in ranges:
        if out and out[-1][1] == lo:
            out[-1][1] = hi
        else:
            out.append([lo, hi])
    return out


# ----- TileContext with a codegen-safe exit drain ---------------------------
# The stock kernel-tail drain carries one semaphore wait per engine/queue the
# kernel touched; CoreV3 codegen rejects instructions with more than two
# waits ("Too many sync wait commands").  Split the waits across preceding
# sync-engine nops, two per instruction, so the drain itself needs none.
class _TileContext(tile.TileContext):
    def _drain_and_barrier(self, tick_clock, wait_clock):
        from concourse.vector_clock import ScopedClock
        nc = self.nc
        probe = nc.sync.nop()
        wait_clock.add_sem_waits(
            probe.ins, ScopedClock({None: tick_clock.global_clock}))
        si = probe.ins.sync_info
        waits = list(si.on_wait) if si and si.on_wait else []
        if len(waits) > 1:
            probe.ins.sync_info = mybir.SyncInfo(
                on_wait=waits[:1], on_update=list(si.on_update or []))
            for w in waits[1:]:
                n = nc.sync.nop()
                n.ins.sync_info = mybir.SyncInfo(on_wait=[w], on_update=[])
        nc.sync.drain()
        nc.all_engine_barrier()
        assert self.sems is not None
        popped = nc._tile_sem_poison_stack.pop()
        assert popped is self._sem_poison
        nc.clear_and_free_semaphores(list(self.sems.allocated().values()))
        nc.all_engine_barrier()


# The same wait-count limit applies to ordinary engine instructions under
# this walrus build, so after the program is fully built, hoist all but one
# wait of every instruction onto preceding same-engine no-ops.
def _legalize_waits(nc, limit=1):
    for bb in nc.main_func.blocks:
        insts = list(bb.instructions)
        out = []
        for inst in insts:
            si = inst.sync_info
            waits = list(si.on_wait) if si and si.on_wait else []
            if len(waits) > limit:
                for w in waits[:-limit]:
                    nop = mybir.InstNoOp(
                        name=nc.get_next_instruction_name(), ins=[], outs=[])
                    nop.engine = inst.engine
                    nop.sync_info = mybir.SyncInfo(on_wait=[w], on_update=[])
                    nc.register_instruction(nop, overwrite=True)
                    out.append(nop)
                inst.sync_info = mybir.SyncInfo(
                    on_wait=waits[-limit:],
                    on_update=list(si.on_update or []))
            out.append(inst)
        bb.instructions = out


DEBUG_TAPS = False


# ----- the bass program -----------------------------------------------------
def build_program(plan, n_pat):
    nc = bass.Bass(num_devices=NCORES)

    xqT = nc.dram_tensor("xqT", [D, S], F16, kind="ExternalInput")
    xkT = nc.dram_tensor("xkT", [D, S], F16, kind="ExternalInput")
    xvT = nc.dram_tensor("xvT", [D, S], F16, kind="ExternalInput")
    wqT = nc.dram_tensor("wqT", [D, EL], F16, kind="ExternalInput")
    wkT = nc.dram_tensor("wkT", [D, EL], F16, kind="ExternalInput")
    wvT = nc.dram_tensor("wvT", [D, EL], F16, kind="ExternalInput")
    woT = nc.dram_tensor("woT", [EL, D], F16, kind="ExternalInput")
    bq2 = nc.dram_tensor("bq2", [2, P], F32, kind="ExternalInput")
    bk2 = nc.dram_tensor("bk2", [2, P], F32, kind="ExternalInput")
    ybias = nc.dram_tensor("ybias", [NMT, P], F32, kind="ExternalInput")
    pats = nc.dram_tensor("pats", [n_pat, KT, KT], F16, kind="ExternalInput")
    yT = nc.dram_tensor("yT", [D, S], F32, kind="ExternalOutput")
    taps = {}
    if DEBUG_TAPS:
        taps["dQt"] = nc.dram_tensor("dQt", [P, 2, S], F16, kind="ExternalOutput")
        taps["dKt"] = nc.dram_tensor("dKt", [P, 2, S], F16, kind="ExternalOutput")
        taps["dVa"] = nc.dram_tensor("dVa", [P, NKT, HL, DK + 1], F16,
                                     kind="ExternalOutput")
        taps["dPt"] = nc.dram_tensor("dPt", [P, NKT, QC], F16,
                                     kind="ExternalOutput")
        taps["dAv"] = nc.dram_tensor("dAv", [P, QC], F32, kind="ExternalOutput")
        taps["dBc"] = nc.dram_tensor("dBc", [P, QC], F32, kind="ExternalOutput")
        taps["dXt"] = nc.dram_tensor("dXt", [P, 2, QC], F16, kind="ExternalOutput")

    with ExitStack() as ctx:
        tc = ctx.enter_context(_TileContext(nc))
        singles = ctx.enter_context(tc.tile_pool(name="singles", bufs=1))

        # --- persistent SBUF state ---
        wq_sb = singles.tile([P, 8, EL], F16)
        wk_sb = singles.tile([P, 8, EL], F16)
        wv_sb = singles.tile([P, 8, EL], F16)
        wo_sb = singles.tile([P, 2, D], F16)
        nc.sync.dma_start(out=wq_sb[:], in_=wqT.rearrange("(a p) e -> p a e", p=P))
        nc.sync.dma_start(out=wk_sb[:], in_=wkT.rearrange("(a p) e -> p a e", p=P))
        nc.sync.dma_start(out=wv_sb[:], in_=wvT.rearrange("(a p) e -> p a e", p=P))
        nc.sync.dma_start(out=wo_sb[:], in_=woT.rearrange("(a p) m -> p a m", p=P))
        bq_sb = singles.tile([P, 2], F32)
        bk_sb = singles.tile([P, 2], F32)
        yb_sb = singles.tile([P, NMT], F32)
        nc.sync.dma_start(out=bq_sb[:], in_=bq2.rearrange("a p -> p a"))
        nc.sync.dma_start(out=bk_sb[:], in_=bk2.rearrange("a p -> p a"))
        nc.sync.dma_start(out=yb_sb[:], in_=ybias.rearrange("a p -> p a"))
        pat_sb = singles.tile([P, n_pat, KT], F16)
        # touch Exp+Ln early so the activation table set loads during the
        # projection phase instead of stalling the first softmax
        warm = singles.tile([P, 1], F32)
        nc.scalar.activation(out=warm[0:1, :], in_=bq_sb[0:1, 0:1],
                             func=mybir.ActivationFunctionType.Exp)
        nc.scalar.activation(out=warm[0:1, :], in_=warm[0:1, :],
                             func=mybir.ActivationFunctionType.Ln)
        nc.sync.dma_start(out=pat_sb[:], in_=pats.rearrange("n p k -> p n k"))

        Qt = singles.tile([P, 2, S], F16)     # [e-within-tile, e-tile, t]
        Kt = singles.tile([P, 2, S], F16)
        Vaug = singles.tile([P, NKT, HL, DK + 1], F16)  # [t-in-ktile, kt, h, e|1]
        nc.vector.memset(Vaug[:, :, :, DK:DK + 1], 1.0)
        ones_row = singles.tile([P, DK], F16)
        nc.vector.memset(ones_row[0:1, :], 1.0)

        # --- phase 1: projections ---
        with tc.tile_pool(name="xin", bufs=4) as xin, \
             tc.tile_pool(name="pjps", bufs=2, space="PSUM") as pjps:
            for name, xdr, w_sb in (("q", xqT, wq_sb), ("k", xkT, wk_sb),
                                    ("v", xvT, wv_sb)):
                xr = xdr.rearrange("(a p) t -> p a t", p=P)
                for tci in range(NQC):
                    tsl = slice(tci * QC, (tci + 1) * QC)
                    x_ch = xin.tile([P, 8, QC], F16, tag="xch", name=f"x_{name}{tci}")
                    nc.sync.dma_start(out=x_ch[:], in_=xr[:, :, tsl])
                    if name != "v":
                        dst, b_sb = (Qt, bq_sb) if name == "q" else (Kt, bk_sb)
                        for et in range(2):
                            ps = pjps.tile([P, QC], F32, tag="pj", name=f"ps_{name}{tci}{et}")
                            for ft in range(8):
                                nc.tensor.matmul(
                                    ps[:],
                                    lhsT=w_sb[:, ft, et * P:(et + 1) * P],
                                    rhs=x_ch[:, ft, :],
                                    start=(ft == 0), stop=(ft == 7))
                            nc.vector.tensor_scalar_add(
                                out=dst[:, et, tsl], in0=ps[:],
                                scalar1=b_sb[:, et:et + 1])
                    else:
                        for tt in range(QC // KT):
                            ktg = tci * (QC // KT) + tt
                            ps = pjps.tile([P, EL], F32, tag="pj", name=f"ps_v{ktg}")
                            for ft in range(8):
                                nc.tensor.matmul(
                                    ps[:],
                                    lhsT=x_ch[:, ft, tt * P:(tt + 1) * P],
                                    rhs=wv_sb[:, ft, :],
                                    start=(ft == 0), stop=(ft == 7))
                            for h in range(HL):
                                nc.vector.tensor_copy(
                                    out=Vaug[:, ktg, h, 0:DK],
                                    in_=ps[:, h * DK:(h + 1) * DK])

        # --- phase 2: attention + output projection, per query chunk ---
        spool = ctx.enter_context(tc.tile_pool(name="spool", bufs=2, space="PSUM"))
        avy = ctx.enter_context(tc.tile_pool(name="avy", bufs=2, space="PSUM"))
        ypp = ctx.enter_context(tc.tile_pool(name="ypp", bufs=2, space="PSUM"))
        ptp = ctx.enter_context(tc.tile_pool(name="ptp", bufs=3))
        xtp = ctx.enter_context(tc.tile_pool(name="xtp", bufs=2))
        nrm = ctx.enter_context(tc.tile_pool(name="nrm", bufs=3))
        ysb = ctx.enter_context(tc.tile_pool(name="ysb", bufs=3))
        dbp = ctx.enter_context(tc.tile_pool(name="dbp", bufs=2, space="DRAM"))

        if DEBUG_TAPS:
            nc.sync.dma_start(out=taps["dQt"][:], in_=Qt[:])
            nc.sync.dma_start(out=taps["dKt"][:], in_=Kt[:])
            nc.sync.dma_start(out=taps["dVa"][:], in_=Vaug[:])

        # output projection for chunk qc, emitted one head into chunk qc+1 so
        # the tensor engine never stalls on the normalize chain of chunk qc
        yTr = yT.rearrange("(a p) t -> p a t", p=P)

        def emit_yproj(qc, xTt):
            for mt in range(NMT):
                yp = ypp.tile([P, QC], F32, tag="yp", name=f"yp{qc}{mt}")
                for ct in range(2):
                    nc.tensor.matmul(
                        yp[:],
                        lhsT=wo_sb[:, ct, mt * P:(mt + 1) * P],
                        rhs=xTt[:, ct, :],
                        start=(ct == 0), stop=(ct == 1))
                ys = ysb.tile([P, QC], F32, tag="ys", name=f"ys{qc}{mt}")
                nc.vector.tensor_scalar_add(out=ys[:], in0=yp[:],
                                            scalar1=yb_sb[:, mt:mt + 1])
                nc.sync.dma_start(out=yTr[:, mt, qc * QC:(qc + 1) * QC],
                                  in_=ys[:])

        pending = None        # (qc, xTt) whose y-projection is not yet emitted
        for qc in range(NQC):
            tiles = plan[qc]
            xTt = xtp.tile([P, 2, QC], F16, tag="xT", name=f"xT{qc}")
            def normalize(h, av):
                # xT_h = av[e] / av[ones-row]; 1/denom = exp(-ln(denom)) on
                # ScalarE — same activation table set as the softmax exp.
                # av is copied out of PSUM right away so the accumulator
                # slot frees for the next head pair without waiting on the
                # broadcast round-trip.
                et = h // 2
                cp = nrm.tile([P, QC], F32, tag="cp", name=f"cp{qc}{h}")
                nc.vector.tensor_copy(out=cp[0:DK, :], in_=av[0:DK, :])
                bc = nrm.tile([P, QC], F32, tag="bc", name=f"bc{qc}{h}")
                rc = nrm.tile([P, QC], F32, tag="rc", name=f"rc{qc}{h}")
                nc.scalar.activation(out=rc[DK:DK + 1, :],
                                     in_=av[DK:DK + 1, :],
                                     func=mybir.ActivationFunctionType.Ln)
                nc.scalar.activation(out=bc[DK:DK + 1, :],
                                     in_=rc[DK:DK + 1, :],
                                     func=mybir.ActivationFunctionType.Exp,
                                     scale=-1.0)
                dnb = dbp.tile([1, QC], F32, tag="dnb", name=f"dnb{qc}{h}")
                nc.sync.dma_start(out=dnb[:], in_=bc[DK:DK + 1, :])
                nc.sync.dma_start(out=bc[0:DK, :],
                                  in_=dnb[0:1, :].partition_broadcast(DK))
                if h % 2 == 0:
                    nc.vector.tensor_tensor(
                        out=xTt[0:DK, et, :], in0=cp[0:DK, :], in1=bc[0:DK, :],
                        op=mybir.AluOpType.mult)
                else:
                    tmp = nrm.tile([P, QC], F16, tag="tmp", name=f"tm{qc}{h}")
                    nc.vector.tensor_tensor(
                        out=tmp[0:DK, :], in0=cp[0:DK, :], in1=bc[0:DK, :],
                        op=mybir.AluOpType.mult)
                    nc.sync.dma_start(out=xTt[DK:P, et, :], in_=tmp[0:DK, :])

            for hp in range(HL // 2):
                # head pair (2hp, 2hp+1): head parity picks SBUF partition
                # half, so the two heads' score matmuls land on different
                # PE row-tiles (T0/T8) and can run concurrently.  One psum
                # tile per k-tile holds both heads side by side; the
                # previous k-tile's A@V matmuls run under the exp latency.
                if hp == 1 and pending is not None:
                    emit_yproj(*pending)
                    pending = None
                et = hp
                ptb = ptp.tile([P, NKT, 2, QC], F16, tag="pt",
                               name=f"pt{qc}{hp}")
                ptbf = ptb.rearrange("p a b c -> p (a b c)")
                avs = [avy.tile([P, QC], F32, tag="avy",
                                name=f"av{qc}{2 * hp + hh}")
                       for hh in range(2)]

                def emit_av(ti, t):
                    for hh in range(2):
                        nc.tensor.matmul(
                            avs[hh][0:DK + 1, t.s0:t.s1],
                            lhsT=Vaug[:, t.kt, 2 * hp + hh, :],
                            rhs=ptb[:, ti, hh, t.s0:t.s1],
                            start=t.first, stop=t.last,
                            skip_group_check=True)

                for ti, t in enumerate(tiles):
                    ps = spool.tile([P, 2 * QC], F32, tag="s",
                                    name=f"s{qc}{hp}{ti}")
                    for hh in range(2):
                        po = hh * DK
                        nc.tensor.matmul(
                            ps[:, hh * QC + t.s0:hh * QC + t.s1],
                            lhsT=Kt[po:po + DK, et,
                                    t.kt * KT:(t.kt + 1) * KT],
                            rhs=Qt[po:po + DK, et,
                                   qc * QC + t.s0:qc * QC + t.s1],
                            start=True, stop=True)
                    if ti > 0:
                        emit_av(ti - 1, tiles[ti - 1])
                    # psum col (hh*QC + c) maps to ptb flat col
                    # (ti*2*QC + hh*QC + c): merged ranges stay merged
                    rgs = _merge_ranges([(hh * QC + t.s0, hh * QC + t.s1)
                                         for hh in range(2)])
                    for lo, hi in rgs:
                        nc.scalar.activation(
                            out=ptbf[:, ti * 2 * QC + lo:ti * 2 * QC + hi],
                            in_=ps[:, lo:hi],
                            func=mybir.ActivationFunctionType.Exp,
                            scale=float(SCALE))
                    for hh in range(2):
                        for qt, pid in t.muls:
                            sl = slice(qt * KT, (qt + 1) * KT)
                            nc.vector.tensor_tensor(
                                out=ptb[:, ti, hh, sl],
                                in0=ptb[:, ti, hh, sl],
                                in1=pat_sb[:, pid, :],
                                op=mybir.AluOpType.mult)
                emit_av(len(tiles) - 1, tiles[-1])
                for hh in range(2):
                    normalize(2 * hp + hh, avs[hh])

            if DEBUG_TAPS and qc == 0:
                nc.sync.dma_start(out=taps["dXt"][:], in_=xTt[:])

            pending = (qc, xTt)
        emit_yproj(*pending)

    _legalize_waits(nc)
    return nc


# ----- SPMD runner ----------------------------------------------------------
# run_bass_kernel_spmd's axon path lowers through jax.jit(shard_map(...)),
# which this jax version emits as `call`-indirect HLO that the bass_exec
# compile hook rejects, and a single 8-replica launch isn't reachable from
# here.  Instead: one single-device jit per core (clean single-computation
# HLO), dispatched asynchronously on all 8 cores.  The NEFF is memoized by
# HLO bytes so walrus runs once, not 8 times.
_NEFF_MEMO = {}


def _install_memo_hook():
    import libneuronxla
    from concourse.bass2jax import install_neuronx_cc_hook

    install_neuronx_cc_hook()
    inner = libneuronxla.neuronx_cc
    if getattr(inner, "_is_memo_hook", False):
        return

    def memo_hook(code, code_format, platform_version, file_prefix):
        import hashlib
        key = hashlib.sha256(bytes(code)).hexdigest()
        if key not in _NEFF_MEMO:
            _NEFF_MEMO[key] = inner(code, code_format, platform_version,
                                    file_prefix)
        return _NEFF_MEMO[key]

    memo_hook._is_memo_hook = True
    libneuronxla.neuronx_cc = memo_hook


def run_spmd(nc, in_maps):
    import jax
    from concourse.bass2jax import _bass_exec_p

    _install_memo_hook()
    n_cores = len(in_maps)
    partition_name = (nc.partition_id_tensor.name
                      if nc.partition_id_tensor is not None else None)
    in_names, out_names, out_avals = [], [], []
    for alloc in nc.m.functions[0].allocations:
        if not isinstance(alloc, mybir.MemoryLocationSet):
            continue
        name = alloc.memorylocations[0].name
        if alloc.kind == "ExternalInput":
            if name != partition_name:
                in_names.append(name)
        elif alloc.kind == "ExternalOutput":
            out_names.append(name)
            out_avals.append(jax.core.ShapedArray(
                tuple(alloc.tensor_shape), mybir.dt.np(alloc.dtype)))
    bind_in_names = tuple(in_names +
                          ([partition_name] if partition_name else []))

    def _body(*args):
        return tuple(_bass_exec_p.bind(
            *args, out_avals=tuple(out_avals), in_names=bind_in_names,
            out_names=tuple(out_names), lowering_input_output_aliases=(),
            sim_require_finite=True, sim_require_nnan=True, nc=nc))

    devices = jax.devices()[:n_cores]
    f = jax.jit(_body)
    futs = []
    for c in range(n_cores):
        args = [jax.device_put(np.asarray(in_maps[c][nm]), devices[c])
                for nm in in_names]
        if partition_name:
            args.append(jax.device_put(np.array([[c]], np.uint32), devices[c]))
        futs.append(f(*args))
    return [{nm: np.asarray(futs[c][i]) for i, nm in enumerate(out_names)}
            for c in range(n_cores)]


# ----- host wrapper ---------------------------------------------------------
_CACHE = {}


def _get_program(mask):
    key = mask.tobytes()
    if key not in _CACHE:
        plan, pats = _mask_plan(mask)
        nc = build_program(plan, pats.shape[0])
        _CACHE[key] = (nc, pats)
    return _CACHE[key]


def make_in_maps(q, k, v, mask, wq, bq, wk, bk, wv, bv, wo, bo, pats):
    q, k, v = (np.asarray(a, np.float32) for a in (q, k, v))
    in_maps = []
    for c in range(NCORES):
        b, g = divmod(c, TP)
        sl = slice(g * EL, (g + 1) * EL)
        woT_g = np.ascontiguousarray(wo[:, sl].T)        # [EL, D]
        in_maps.append({
            "xqT": np.ascontiguousarray(q[b].T.astype(F16NP)),
            "xkT": np.ascontiguousarray(k[b].T.astype(F16NP)),
            "xvT": np.ascontiguousarray(v[b].T.astype(F16NP)),
            "wqT": np.ascontiguousarray(wq[sl, :].T.astype(F16NP)),
            "wkT": np.ascontiguousarray(wk[sl, :].T.astype(F16NP)),
            "wvT": np.ascontiguousarray(wv[sl, :].T.astype(F16NP)),
            "woT": woT_g.astype(F16NP),
            "bq2": np.ascontiguousarray(bq[sl].reshape(2, P)),
            "bk2": np.ascontiguousarray(bk[sl].reshape(2, P)),
            "ybias": np.ascontiguousarray(
                (bv[sl].astype(np.float64) @ woT_g.astype(np.float64))
                .astype(np.float32).reshape(NMT, P)),
            "pats": pats,
        })
    return in_maps


def assemble_output(results, bo):
    y = np.empty((B, S, D), np.float32)
    for b in range(B):
        acc = results[b * TP]["yT"].astype(np.float32)
        for g in range(1, TP):
            acc = acc + results[b * TP + g]["yT"]
        y[b] = acc.T + np.asarray(bo, np.float32)[None, :]
    return y


def kernel(q, k, v, mask, wq, bq, wk, bk, wv, bv, wo, bo):
    mask2d = np.asarray(mask).reshape(S, S)
    nc, pats = _get_program(mask2d)
    in_maps = make_in_maps(q, k, v, mask2d, wq, bq, wk, bk, wv, bv, wo, bo, pats)
    return assemble_output(run_spmd(nc, in_maps), bo)

